# revision 2
# baseline (speedup 1.0000x reference)
"""Trainium2 Bass kernel for the 2-layer LSTM bar decoder.

Model (per bar, 16 bars, all sharing weights):
  16 steps of: x = [out, emb]; (h0,c0)=LSTMCell0(x); (h1,c1)=LSTMCell1(h0);
  out = softmax(h1 @ W_out.T + b_out)

Strategy:
  - Data-parallel over (bar, batch): 16 bars x 256 batch = 4096 independent
    rows; each of 8 cores owns a 32-batch slice x all bars = 512 rows.
  - All state kept TRANSPOSED in SBUF ([hidden, rows], hidden on partitions)
    so every matmul uses the small weights as the stationary operand and the
    512-row state as the moving operand (N=512, full PE efficiency).
  - fp16 matmul operands (1 cycle/row on PE vs 4 for fp32; fp32 PSUM accum).
  - emb contribution of layer-0 gates is step-invariant: computed once,
    added per step with one DVE add per gate tile.
  - sigmoid built from tanh (sigma(x) = 0.5 + 0.5*tanh(x/2)) so the whole
    kernel needs a single ACT table set (exp_and_others: tanh + exp).
  - softmax over hidden (= partition dim here): column sums via ones-matmul,
    reciprocal on DVE, broadcast back via a K=1 ones-matmul; final out tile
    transposed back to [rows, hidden] on the PE and DMA'd straight from PSUM.
"""

import os
import sys

import numpy as np

H = 512
BARS = 16
UNITS = 16
B = 256
NCORES = 8
BPC = B // NCORES  # batch rows per core
R = BARS * BPC  # rows per core (bar-major)
GT = (4 * H) // 128  # gate tiles per layer
KT = H // 128  # k (hidden) tiles
RT = R // 128  # row tiles

LAST_EXEC_NS = None

_cache = {}


def _ensure_path():
    for p in ("/opt/trn_rl_repo",):
        if os.path.isdir(p) and p not in sys.path:
            sys.path.insert(0, p)


def _build_nc():
    _ensure_path()
    import concourse.tile as tile
    from concourse import bacc, mybir
    from concourse.masks import make_identity

    f16 = mybir.dt.float16
    f32 = mybir.dt.float32
    AF = mybir.ActivationFunctionType
    ALU = mybir.AluOpType

    nc = bacc.Bacc("TRN2")

    w0a = nc.declare_dram_parameter("w0a", [H, 4 * H], f16, isOutput=False)
    w0b = nc.declare_dram_parameter("w0b", [H, 4 * H], f16, isOutput=False)
    w0h = nc.declare_dram_parameter("w0h", [H, 4 * H], f16, isOutput=False)
    w1i = nc.declare_dram_parameter("w1i", [H, 4 * H], f16, isOutput=False)
    w1h = nc.declare_dram_parameter("w1h", [H, 4 * H], f16, isOutput=False)
    wo = nc.declare_dram_parameter("wo", [H, H], f16, isOutput=False)
    b0 = nc.declare_dram_parameter("b0", [128, GT], f32, isOutput=False)
    b1h = nc.declare_dram_parameter("b1h", [128, GT], f32, isOutput=False)
    b1f = nc.declare_dram_parameter("b1f", [128, GT], f32, isOutput=False)
    bo = nc.declare_dram_parameter("bo", [128, KT], f32, isOutput=False)
    embT = nc.declare_dram_parameter("embT", [H, R], f16, isOutput=False)
    h0T = nc.declare_dram_parameter("h0T", [H, R], f16, isOutput=False)
    h1T = nc.declare_dram_parameter("h1T", [H, R], f16, isOutput=False)
    c0T = nc.declare_dram_parameter("c0T", [H, R], f16, isOutput=False)
    c1T = nc.declare_dram_parameter("c1T", [H, R], f16, isOutput=False)
    oT = nc.declare_dram_parameter("oT", [H, R], f16, isOutput=False)
    out = nc.declare_dram_parameter("out", [BPC, BARS * UNITS, H], f32, isOutput=True)

    # [b, bar*16+u, h] viewed as [bar, u, b, h] for per-(step,row-tile) stores
    out_v = out[:, :, :].rearrange("b (bar u) h -> bar u b h", bar=BARS)

    with tile.TileContext(nc) as tc:
        with (
            tc.tile_pool(name="consts", bufs=1) as consts,
            tc.tile_pool(name="wpool", bufs=1) as wpool,
            tc.tile_pool(name="eppool", bufs=1) as eppool,
            tc.tile_pool(name="cpool", bufs=1) as cpool,
            tc.tile_pool(name="hpool", bufs=2) as hpool,
            tc.tile_pool(name="gsb", bufs=1) as gsb,
            tc.tile_pool(name="cellsb", bufs=2) as cellsb,
            tc.tile_pool(name="smx", bufs=2) as smx,
            tc.tile_pool(name="pg", bufs=3, space="PSUM") as pg,
            tc.tile_pool(name="plog", bufs=2, space="PSUM") as plog,
            tc.tile_pool(name="psb", bufs=1, space="PSUM") as psb,
            tc.tile_pool(name="ptr", bufs=2, space="PSUM") as ptr,
        ):
            ident = consts.tile([128, 128], f16, tag="ident")
            make_identity(nc, ident)
            ones_k = consts.tile([128, 1], f32, tag="ones_k")
            nc.vector.memset(ones_k, 1.0)
            ones_m = consts.tile([1, 128], f32, tag="ones_m")
            nc.vector.memset(ones_m, 1.0)

            # warmup ops with minimal sync waits so the implicit ACT/DVE
            # table loads don't attach to instructions that already carry
            # multiple semaphore waits (walrus sync-wait limit).
            warm = consts.tile([128, 1], f32, tag="warm")
            nc.scalar.activation(warm[:, :], ones_k[:, :], AF.Tanh)
            nc.scalar.activation(warm[:, :], warm[:, :], AF.Exp)
            nc.vector.reciprocal(warm[:, :], warm[:, :])

            b0_sb = consts.tile([128, GT], f32, tag="b0")
            nc.sync.dma_start(out=b0_sb, in_=b0[:, :])
            b1h_sb = consts.tile([128, GT], f32, tag="b1h")
            nc.sync.dma_start(out=b1h_sb, in_=b1h[:, :])
            b1f_sb = consts.tile([128, GT], f32, tag="b1f")
            nc.sync.dma_start(out=b1f_sb, in_=b1f[:, :])
            bo_sb = consts.tile([128, KT], f32, tag="bo")
            nc.sync.dma_start(out=bo_sb, in_=bo[:, :])

            def load_ktiles(dram, pool, name, free, dtype):
                ts = []
                for k in range(KT):
                    t = pool.tile([128, free], dtype, tag=f"{name}{k}")
                    nc.sync.dma_start(out=t[:, :], in_=dram[k * 128 : (k + 1) * 128, :])
                    ts.append(t)
                return ts

            w0a_sb = load_ktiles(w0a, wpool, "w0a", 4 * H, f16)
            w0h_sb = load_ktiles(w0h, wpool, "w0h", 4 * H, f16)
            w1i_sb = load_ktiles(w1i, wpool, "w1i", 4 * H, f16)
            w1h_sb = load_ktiles(w1h, wpool, "w1h", 4 * H, f16)
            wo_sb = load_ktiles(wo, wpool, "wo", H, f16)

            cur_h0 = load_ktiles(h0T, hpool, "h0_", R, f16)
            cur_h1 = load_ktiles(h1T, hpool, "h1_", R, f16)
            cur_o = load_ktiles(oT, hpool, "o_", R, f16)
            c0_sb = load_ktiles(c0T, cpool, "c0_", R, f16)
            c1_sb = load_ktiles(c1T, cpool, "c1_", R, f16)

            # -------- precompute: embpre[gt] = (W_ih0_emb @ embT + b0)[gt] --------
            embpre = []
            with tc.tile_pool(name="prepool", bufs=1) as prepool:
                embT_sb = load_ktiles(embT, prepool, "embT", R, f16)
                w0b_sb = load_ktiles(w0b, prepool, "w0b", 4 * H, f16)
                for gt in range(GT):
                    ps = pg.tile([128, R], f32, tag="g")
                    for k in range(KT):
                        nc.tensor.matmul(
                            ps[:, :],
                            w0b_sb[k][:, gt * 128 : (gt + 1) * 128],
                            embT_sb[k][:, :],
                            start=(k == 0),
                            stop=(k == KT - 1),
                        )
                    ep = eppool.tile([128, R], f16, tag=f"ep{gt}")
                    nc.scalar.activation(
                        ep[:, :], ps[:, :], AF.Identity, bias=b0_sb[:, gt : gt + 1]
                    )
                    embpre.append(ep)

            # -------- recurrence --------
            def lstm_layer(w_h, h_old, w_x, x_new, emb_add, bias_half, bias_full, c_sb, htag):
                """One LSTM layer in transposed layout. Returns new h (4 k-tiles)."""
                tg = [None] * GT
                for gt in range(GT):
                    ps = pg.tile([128, R], f32, tag="g")
                    for k in range(KT):
                        nc.tensor.matmul(
                            ps[:, :],
                            w_h[k][:, gt * 128 : (gt + 1) * 128],
                            h_old[k][:, :],
                            start=(k == 0),
                            stop=False,
                        )
                    for k in range(KT):
                        nc.tensor.matmul(
                            ps[:, :],
                            w_x[k][:, gt * 128 : (gt + 1) * 128],
                            x_new[k][:, :],
                            start=False,
                            stop=(k == KT - 1),
                        )
                    if emb_add is not None:
                        nc.vector.tensor_add(ps[:, :], ps[:, :], emb_add[gt][:, :])
                    tgt = gsb.tile([128, R], f16, tag=f"tg{gt}")
                    if 8 <= gt < 12:  # g gate: tanh(x)
                        if bias_full is not None:
                            nc.scalar.activation(
                                tgt[:, :], ps[:, :], AF.Tanh, bias=bias_full[:, gt : gt + 1]
                            )
                        else:
                            nc.scalar.activation(tgt[:, :], ps[:, :], AF.Tanh)
                    else:  # i/f/o gates: tanh(x/2) -> sigmoid
                        if bias_half is not None:
                            nc.scalar.activation(
                                tgt[:, :],
                                ps[:, :],
                                AF.Tanh,
                                bias=bias_half[:, gt : gt + 1],
                                scale=0.5,
                            )
                        else:
                            nc.scalar.activation(tgt[:, :], ps[:, :], AF.Tanh, scale=0.5)
                    tg[gt] = tgt
                new_h = [None] * KT
                for ht in range(KT):
                    ti, tf, tgg, to = tg[ht], tg[4 + ht], tg[8 + ht], tg[12 + ht]
                    # sigma = 0.5*tanh + 0.5, in place
                    nc.vector.tensor_scalar(ti[:, :], ti[:, :], 0.5, 0.5, ALU.mult, ALU.add)
                    nc.vector.tensor_scalar(tf[:, :], tf[:, :], 0.5, 0.5, ALU.mult, ALU.add)
                    nc.vector.tensor_scalar(to[:, :], to[:, :], 0.5, 0.5, ALU.mult, ALU.add)
                    m1 = cellsb.tile([128, R], f16, tag=f"m1_{ht}")
                    nc.vector.tensor_mul(m1[:, :], tf[:, :], c_sb[ht][:, :])
                    nc.vector.tensor_mul(ti[:, :], ti[:, :], tgg[:, :])
                    nc.vector.tensor_add(c_sb[ht][:, :], m1[:, :], ti[:, :])
                    tch = cellsb.tile([128, R], f16, tag=f"tc_{ht}")
                    nc.scalar.activation(tch[:, :], c_sb[ht][:, :], AF.Tanh)
                    nh = hpool.tile([128, R], f16, tag=f"{htag}{ht}")
                    nc.vector.tensor_mul(nh[:, :], to[:, :], tch[:, :])
                    new_h[ht] = nh
                return new_h

            repeat = int(os.environ.get("KREPEAT", "1"))
            for t in list(range(UNITS)) * repeat:
                new_h0 = lstm_layer(
                    w0h_sb, cur_h0, w0a_sb, cur_o, embpre, None, None, c0_sb, "h0_"
                )
                new_h1 = lstm_layer(
                    w1h_sb, cur_h1, w1i_sb, new_h0, None, b1h_sb, b1f_sb, c1_sb, "h1_"
                )

                # logits -> exp
                e_t = [None] * KT
                for mt in range(KT):
                    ps = plog.tile([128, R], f32, tag="lg")
                    for k in range(KT):
                        nc.tensor.matmul(
                            ps[:, :],
                            wo_sb[k][:, mt * 128 : (mt + 1) * 128],
                            new_h1[k][:, :],
                            start=(k == 0),
                            stop=(k == KT - 1),
                        )
                    et = smx.tile([128, R], f32, tag=f"e{mt}")
                    nc.scalar.activation(
                        et[:, :], ps[:, :], AF.Exp, bias=bo_sb[:, mt : mt + 1]
                    )
                    e_t[mt] = et

                # column sums over hidden (partition axis) via ones-matmul
                s = smx.tile([128, R], f32, tag="s")
                nc.vector.tensor_add(s[:, :], e_t[0][:, :], e_t[1][:, :])
                nc.vector.tensor_add(s[:, :], s[:, :], e_t[2][:, :])
                nc.vector.tensor_add(s[:, :], s[:, :], e_t[3][:, :])
                ps_sum = psb.tile([1, R], f32, tag="sb")
                nc.tensor.matmul(ps_sum[:, :], ones_k[:, :], s[:, :], start=True, stop=True)
                rec = cellsb.tile([1, R], f32, tag="rec")
                nc.vector.reciprocal(rec[:, :], ps_sum[:, :])
                ps_b = psb.tile([128, R], f32, tag="sb")
                nc.tensor.matmul(ps_b[:, :], ones_m[:, :], rec[:, :], start=True, stop=True)

                new_o = [None] * KT
                for mt in range(KT):
                    no = hpool.tile([128, R], f16, tag=f"o_{mt}")
                    nc.vector.tensor_mul(no[:, :], e_t[mt][:, :], ps_b[:, :])
                    new_o[mt] = no

                # transpose back to [rows, hidden] and store
                for rt in range(RT):
                    pst = ptr.tile([128, H], f16, tag="tr")
                    for hc in range(KT):
                        nc.tensor.transpose(
                            pst[:, hc * 128 : (hc + 1) * 128],
                            new_o[hc][:, rt * 128 : (rt + 1) * 128],
                            ident[:, :],
                        )
                    stg = smx.tile([128, H], f32, tag="stg")
                    nc.any.tensor_copy(stg[:, :], pst[:, :])
                    nc.sync.dma_start(
                        out=out_v[rt * 4 : (rt + 1) * 4, t, :, :], in_=stg[:, :]
                    )

                cur_h0, cur_h1, cur_o = new_h0, new_h1, new_o

    return nc


def _get_nc():
    if "nc" not in _cache:
        nc = _build_nc()
        if not nc.is_finalized():
            nc.finalize()
        _cache["nc"] = nc
    return _cache["nc"]


def _make_in_maps(inputs):
    x = {k: np.asarray(v) for k, v in inputs.items()}
    W_ih0 = x["W_ih0"].astype(np.float32)
    W_hh0 = x["W_hh0"].astype(np.float32)
    W_ih1 = x["W_ih1"].astype(np.float32)
    W_hh1 = x["W_hh1"].astype(np.float32)
    W_out = x["W_out"].astype(np.float32)
    b0 = (x["b_ih0"] + x["b_hh0"]).astype(np.float32)
    b1 = (x["b_ih1"] + x["b_hh1"]).astype(np.float32)
    b_out = x["b_out"].astype(np.float32)
    emb = x["embedding_C"].astype(np.float32)
    h0 = x["h0"].astype(np.float32)
    c0 = x["c0"].astype(np.float32)
    out0 = x["out0"].astype(np.float32)

    shared = {
        "w0a": np.ascontiguousarray(W_ih0[:, :H].T).astype(np.float16),
        "w0b": np.ascontiguousarray(W_ih0[:, H:].T).astype(np.float16),
        "w0h": np.ascontiguousarray(W_hh0.T).astype(np.float16),
        "w1i": np.ascontiguousarray(W_ih1.T).astype(np.float16),
        "w1h": np.ascontiguousarray(W_hh1.T).astype(np.float16),
        "wo": np.ascontiguousarray(W_out.T).astype(np.float16),
        "b0": np.ascontiguousarray(b0.reshape(GT, 128).T),
        "b1h": np.ascontiguousarray((b1 * 0.5).reshape(GT, 128).T),
        "b1f": np.ascontiguousarray(b1.reshape(GT, 128).T),
        "bo": np.ascontiguousarray(b_out.reshape(KT, 128).T),
    }

    def t16(rows_by_h):  # [R, H] -> [H, R] fp16
        return np.ascontiguousarray(rows_by_h.T).astype(np.float16)

    in_maps = []
    for c in range(NCORES):
        bs = slice(c * BPC, (c + 1) * BPC)
        m = dict(shared)
        m["embT"] = t16(np.swapaxes(emb[bs], 0, 1).reshape(R, H))
        m["h0T"] = t16(h0[:, 0, bs, :].reshape(R, H))
        m["h1T"] = t16(h0[:, 1, bs, :].reshape(R, H))
        m["c0T"] = t16(c0[:, 0, bs, :].reshape(R, H))
        m["c1T"] = t16(c0[:, 1, bs, :].reshape(R, H))
        m["oT"] = t16(out0[:, bs, :].reshape(R, H))
        in_maps.append(m)
    return in_maps


def kernel(**inputs):
    global LAST_EXEC_NS
    _ensure_path()
    from concourse.bass_utils import run_bass_kernel_spmd

    in_maps = _make_in_maps(inputs)
    nc = _get_nc()
    trace = bool(os.environ.get("KTRACE"))
    kw = {}
    if trace and os.environ.get("KTRACE_DIR"):
        os.makedirs(os.environ["KTRACE_DIR"], exist_ok=True)
        kw["tmpdir"] = os.environ["KTRACE_DIR"]
    try:
        res = run_bass_kernel_spmd(nc, in_maps, list(range(NCORES)), trace=trace, **kw)
    except (ImportError, ModuleNotFoundError):
        res = run_bass_kernel_spmd(nc, in_maps, list(range(NCORES)), trace=False)
    if getattr(res, "exec_time_ns", None):
        LAST_EXEC_NS = res.exec_time_ns

    outs = [np.asarray(res.results[c]["out"], dtype=np.float32) for c in range(NCORES)]
    return np.concatenate(outs, axis=0)


if __name__ == "__main__":
    nc = _get_nc()
    print("built ok")

